# revision 46
# baseline (speedup 1.0000x reference)
"""Trainium2 Bass kernel for linear multi-head attention (elu+1 feature map).

Math (per batch n):
  q = x_q @ Wq.T ; k = x_k @ Wk.T ; v = (x_v @ Wv.T) / L
  Q = elu(q)+1 ; K = elu(k)+1
  KV[h] = K_h.T @ v_h              (D x D per head)
  Ksum  = sum_s K[s, :]            (E)
  S[l,h] = Q_h[l] . Ksum_h ;  W = L / (S + eps)
  msg[l, h*D+dv] = (Q_h[l] @ KV[h])[dv] * W[l,h]
  out = msg @ Wm.T

Sharding: B*L = 16384 rows split into 8 chunks of 2048 (each core gets half
of one batch's sequence). Only cross-core dependency: the KV/Ksum reduction
between the two cores sharing a batch -> pairwise AllReduce (f32, 66KB).

Final structure (218.9us baseline -> 115.9us):
  - inputs/weights cast to bf16 on the host: halves HBM traffic + SBUF and
    enables fast weight loads (FWL); PSUM accumulation stays f32.
  - all input DMA on the sync (SP) HWDGE queue, phase-A tensors first
    (one DMA per 512-row stripe across all 4 k-blocks), q-side after.
  - engine warmups at t=0: dummy matmuls trip the HAM clock gate to
    K=8/8 and a dummy Exp pulls in the ACT table load; a tiny warm-up
    AllGather absorbs the ~11.5us first-collective ncfw dispatch cost.
  - KV cross-product is group-local (K_g^T @ V_g, N=128) packed into a
    single PSUM bank; Ksum kept in partition layout via N=1 matmuls;
    2-tile software-pipeline skew hides the elu chain.
  - pairwise bf16 AllGather of the packed [kv|ksum] partials + one local
    DVE add (faster than AllReduce; an 8-rank group measured worse).
  - S computed directly in replicated layout ([128, CH] per group) by
    matmul with a block-masked Ksum operand -> no partition-broadcast
    DMA; W = 1/(S/L+eps/L) via ACT scale/bias + DVE fast reciprocal.
  - phase B: per-chunk S/msg matmuls with merges interleaved one chunk
    behind; last chunk emits all S matmuls first to shorten the tail.
"""

import numpy as np

B = 4
L = 4096
E = 512
H = 16
D = 32
P = 128
KT = E // P
NCORES = 8
R = (B * L) // NCORES
ST = R // P
NCHUNK = 4
CH = R // NCHUNK
EPS = 1e-6
CW = P + KT          # packed collective row: 128 kv cols + 4 ksum cols
CC = P * CW

_CACHE = {}
LAST_EXEC_NS = None
LAST_RESULTS = None


def _build():
    import concourse.bass as bass
    import concourse.mybir as mybir
    import concourse.tile as tile
    from concourse import bacc

    f32 = mybir.dt.float32
    bf16 = mybir.dt.bfloat16
    AFT = mybir.ActivationFunctionType
    OP = mybir.AluOpType

    nc = bacc.Bacc("TRN2", target_bir_lowering=False, debug=False,
                   num_devices=NCORES)

    xq_d = nc.dram_tensor("xq", [E, R], bf16, kind="ExternalInput").ap()
    xk_d = nc.dram_tensor("xk", [E, R], bf16, kind="ExternalInput").ap()
    xv_d = nc.dram_tensor("xv", [E, R], bf16, kind="ExternalInput").ap()
    wq_d = nc.dram_tensor("wq", [E, E], bf16, kind="ExternalInput").ap()
    wk_d = nc.dram_tensor("wk", [E, E], bf16, kind="ExternalInput").ap()
    wv_d = nc.dram_tensor("wv", [E, E], bf16, kind="ExternalInput").ap()
    wm_d = nc.dram_tensor("wm", [E, E], bf16, kind="ExternalInput").ap()
    out_d = nc.dram_tensor("out", [R, E], f32, kind="ExternalOutput").ap()

    RG = [[0, 1], [2, 3], [4, 5], [6, 7]]

    with tile.TileContext(nc) as tc:

        with tc.tile_pool(name="const", bufs=1) as const, \
             tc.tile_pool(name="xq_pool", bufs=1) as xq_pool, \
             tc.tile_pool(name="qt_pool", bufs=1) as qt_pool, \
             tc.tile_pool(name="dram", bufs=1, space="DRAM") as dram:

            wq_sb = const.tile([P, KT, E], bf16)
            # ---- engine warmups: dummy matmuls push HAM to K=8/8 and a
            # dummy Exp pulls the ACT table load off the critical path,
            # all while the input DMA prefill streams.
            warm_sb = const.tile([P, E], bf16)
            nc.vector.memset(warm_sb[:], 0.0)
            warm_f = const.tile([P, 4], f32)
            nc.scalar.activation(warm_f[:], warm_sb[:, 0:4], AFT.Exp)
            wk_sb = const.tile([P, KT, E], bf16)
            wv_sb = const.tile([P, KT, E], bf16)
            wm_sb = const.tile([P, KT, E], bf16)
            ones_f32 = const.tile([P, 1], f32)
            nc.vector.memset(ones_f32[:], 1.0)
            ones_sb = const.tile([P, 1], bf16)
            nc.vector.tensor_copy(ones_sb[:], ones_f32[:])

            # block-identity mask: maskI[k, p] = (k//32 == p//32)
            maskI_np = np.zeros((P, P), np.float32)
            for j in range(4):
                maskI_np[32 * j:32 * (j + 1), 32 * j:32 * (j + 1)] = 1.0
            maskI_d = nc.inline_tensor(maskI_np, name="blk_ident")
            maskI_sb = const.tile([P, P], f32)
            nc.gpsimd.dma_start(maskI_sb[:], maskI_d.ap())

            # tiny warm-up AllGather: absorbs the ~11.5us first-collective
            # ncfw dispatch overhead while phase A streams (result unused)
            ccw_in = dram.tile([16], bf16)
            ccw_out = dram.tile([32], bf16)
            ccw_sb = const.tile([1, 16], bf16)
            nc.vector.memset(ccw_sb[:], 0.0)
            nc.gpsimd.dma_start(
                ccw_in[:].rearrange("(p f) -> p f", p=1), ccw_sb[:])
            nc.gpsimd.collective_compute(
                "AllGather", mybir.AluOpType.bypass, replica_groups=RG,
                ins=[ccw_in[:].opt()], outs=[ccw_out[:].opt()])

            cc_in = dram.tile([CC], bf16)
            cc_out2 = dram.tile([2 * CC], bf16)

            wv_r = wv_d.rearrange("(ko ki) n -> ki ko n", ki=P)
            wk_r = wk_d.rearrange("(ko ki) n -> ki ko n", ki=P)
            wq_r = wq_d.rearrange("(ko ki) n -> ki ko n", ki=P)
            wm_r = wm_d.rearrange("(ko ki) n -> ki ko n", ki=P)
            xv_r = xv_d.rearrange("(ko ki) n -> ki ko n", ki=P)
            xk_r = xk_d.rearrange("(ko ki) n -> ki ko n", ki=P)
            xq_r = xq_d.rearrange("(ko ki) n -> ki ko n", ki=P)

            # =================== Phase A: k/v proj + KV/Ksum ===============
            with tc.tile_pool(name="xkv_pool", bufs=1) as xkv_pool, \
                 tc.tile_pool(name="workA", bufs=4) as workA, \
                 tc.tile_pool(name="psA", bufs=5, space="PSUM") as psA, \
                 tc.tile_pool(name="kvp", bufs=1, space="PSUM") as kvp:

                xk_sb = xkv_pool.tile([P, KT, R], bf16)
                xv_sb = xkv_pool.tile([P, KT, R], bf16)
                xq_sb = xq_pool.tile([P, KT, R], bf16)

                # dummy matmuls on the scratch tile keep the PE busy (and
                # HAM warming) while the first input stripes stream in
                warm_ps = kvp.tile([P, E], f32, name="warm_ps")
                for _ in range(9):
                    nc.tensor.matmul(warm_ps[:], warm_sb[:, 0:P],
                                     warm_sb[:], start=True, stop=True)

                # ---- DMA order on the sync queue (SP engine): k/v weights
                # + first 512-row stripe (first tile's rows split out),
                # rest of k/v stripes, then wq, xq (chunk-major), wm.
                nc.sync.dma_start(wv_sb[:], wv_r)
                nc.sync.dma_start(xv_sb[:, :, 0:P], xv_r[:, :, 0:P])
                nc.sync.dma_start(wk_sb[:], wk_r)
                nc.sync.dma_start(xk_sb[:, :, 0:P], xk_r[:, :, 0:P])
                nc.sync.dma_start(xv_sb[:, :, P:CH], xv_r[:, :, P:CH])
                nc.sync.dma_start(xk_sb[:, :, P:CH], xk_r[:, :, P:CH])
                for sc in range(1, NCHUNK):
                    cs = slice(sc * CH, (sc + 1) * CH)
                    nc.sync.dma_start(xv_sb[:, :, cs], xv_r[:, :, cs])
                    nc.sync.dma_start(xk_sb[:, :, cs], xk_r[:, :, cs])
                nc.sync.dma_start(wq_sb[:], wq_r)
                for c in range(NCHUNK):
                    cs = slice(c * CH, (c + 1) * CH)
                    nc.sync.dma_start(xq_sb[:, :, cs], xq_r[:, :, cs])
                nc.sync.dma_start(wm_sb[:], wm_r)

                # group-local KV accumulator: kvacc[:, g, :] = K_g^T @ V_g
                # (4 x [128,128] side by side = exactly one PSUM bank)
                kvacc = kvp.tile([P, KT, P], f32)
                # Ksum in partition layout: ks_ps[k, g] = sum_s K[s, 128g+k]
                ks_ps = kvp.tile([P, KT], f32)

                # software pipeline: KV(si-2) emitted between projections of
                # si so the PE never waits for the 2.5us elu chain
                SKEW = 2
                kv_q = {}
                for si in range(ST + SKEW):
                    if si < ST:
                        sl = slice(si * P, (si + 1) * P)
                        v_ps = psA.tile([P, E], f32, name="v_ps", tag="proj")
                        for ko in range(KT):
                            nc.tensor.matmul(
                                v_ps[:], xv_sb[:, ko, sl], wv_sb[:, ko, :],
                                start=(ko == 0), stop=(ko == KT - 1))
                        v_sb = workA.tile([P, E], bf16, name="v_sb")
                        nc.scalar.copy(v_sb[:], v_ps[:])

                        k_ps = psA.tile([P, E], f32, name="k_ps", tag="proj")
                        for ko in range(KT):
                            nc.tensor.matmul(
                                k_ps[:], xk_sb[:, ko, sl], wk_sb[:, ko, :],
                                start=(ko == 0), stop=(ko == KT - 1))
                        # elu(x)+1 = Exp(-Relu(-x)) + max(x,0)
                        # Relu step on DVE, Exp on ACT, combine on DVE.
                        tA = workA.tile([P, E], f32, name="tAk", tag="tAk")
                        nc.vector.tensor_scalar(
                            tA[:], k_ps[:], -1.0, 0.0, OP.mult, OP.max)
                        tB = workA.tile([P, E], f32, name="tBk", tag="tBk")
                        nc.scalar.activation(tB[:], tA[:], AFT.Exp,
                                             scale=-1.0)
                        k_sb = workA.tile([P, E], bf16, name="k_sb")
                        nc.vector.scalar_tensor_tensor(
                            k_sb[:], k_ps[:], 0.0, tB[:], OP.max, OP.add)
                        kv_q[si] = (k_sb, v_sb)
                    if si >= SKEW:
                        pk, pv = kv_q.pop(si - SKEW)
                        for g in range(KT):
                            gsl = slice(g * P, (g + 1) * P)
                            # start=True clears has_written for the WHOLE
                            # bank: only the first group may set it.
                            nc.tensor.matmul(
                                kvacc[:, g, :], pk[:, gsl], pv[:, gsl],
                                start=(si == SKEW and g == 0),
                                stop=(si == ST + SKEW - 1))
                            nc.tensor.matmul(
                                ks_ps[:, g:g + 1], pk[:, gsl],
                                ones_sb[:, 0:1],
                                start=(si == SKEW and g == 0),
                                stop=(si == ST + SKEW - 1))

                # pack [kv diag blocks | ksum] into one [P, 132] bf16 tile:
                # 4 strided PSUM->SBUF copies (one per 32-row block) + 1
                # ksum copy, then a single store for the collective.
                kvks_sb = workA.tile([P, CW], bf16, name="kvks_sb")
                kv_view = kvks_sb[:, 0:P].rearrange("p (g f) -> p g f", g=KT)
                for j in range(KT):
                    nc.vector.tensor_copy(
                        kv_view[32 * j:32 * (j + 1), :, :],
                        kvacc[32 * j:32 * (j + 1), :,
                              32 * j:32 * (j + 1)])
                nc.vector.tensor_copy(kvks_sb[:, P:CW], ks_ps[:])
                nc.sync.dma_start(
                    cc_in[:].rearrange("(p f) -> p f", p=P), kvks_sb[:])

            # ============ pairwise AllGather (reduce locally after) =========
            # AG is a pure copy (no CCE reduce), lower latency than
            # AllReduce; a single 8-rank group measured WORSE (all-rank
            # barrier waits on the slowest-launched core).
            nc.gpsimd.collective_compute(
                "AllGather", mybir.AluOpType.bypass, replica_groups=RG,
                ins=[cc_in[:].opt()], outs=[cc_out2[:].opt()])

            # =================== Phase B ====================================
            # q projection + elu for ALL chunks first (overlaps the
            # collective); Q is stored bf16.
            qt_sb = qt_pool.tile([P, KT, R], bf16)
            with tc.tile_pool(name="workQ", bufs=3) as workQ, \
                 tc.tile_pool(name="psQ", bufs=3, space="PSUM") as psQ:
                for c in range(NCHUNK):
                    cs = slice(c * CH, (c + 1) * CH)
                    for no in range(KT):
                        q_ps = psQ.tile([P, CH], f32, name="q_ps")
                        for ko in range(KT):
                            nc.tensor.matmul(
                                q_ps[:], wq_sb[:, ko, no * P:(no + 1) * P],
                                xq_sb[:, ko, cs],
                                start=(ko == 0), stop=(ko == KT - 1))
                        # alternate the Relu step ACT/DVE so neither engine
                        # paces the elu chain during the AllGather window
                        tA = workQ.tile([P, CH], f32, name="tAq", tag="tAq")
                        if no % 2 == 0:
                            nc.scalar.activation(tA[:], q_ps[:], AFT.Relu,
                                                 scale=-1.0)
                        else:
                            nc.vector.tensor_scalar(
                                tA[:], q_ps[:], -1.0, 0.0, OP.mult, OP.max)
                        tB = workQ.tile([P, CH], f32, name="tBq", tag="tBq")
                        nc.scalar.activation(tB[:], tA[:], AFT.Exp,
                                             scale=-1.0)
                        nc.vector.scalar_tensor_tensor(
                            qt_sb[:, no, cs], q_ps[:], 0.0, tB[:],
                            OP.max, OP.add)
                # small bridge of dummy matmuls: pads the PE-idle window
                # while the AllGather completes so HAM stays at K=8/8
                bridge_ps = psQ.tile([P, CH], f32, name="q_ps")
                for _ in range(10):
                    nc.tensor.matmul(bridge_ps[:], warm_sb[:, 0:P],
                                     warm_sb[:], start=True, stop=True)

            # ---- gathered partials -> one local add -> kv / ks
            two = const.tile([P, 2, CW], bf16)
            nc.sync.dma_start(
                two[:], cc_out2[:].rearrange("(r p f) -> p r f", r=2, p=P))
            red = const.tile([P, CW], bf16)
            nc.vector.tensor_tensor(
                red[:], two[:, 0, :], two[:, 1, :], OP.add)
            ks_f32 = const.tile([P, KT], f32)
            nc.vector.tensor_copy(ks_f32[:], red[:, P:CW])
            # bd_rep[:, g, p] = Ksum[128g + k] if k//32 == p//32 else 0
            bd_rep = const.tile([P, KT, P], bf16)
            for g in range(KT):
                nc.vector.tensor_tensor(
                    bd_rep[:, g, :], maskI_sb[:],
                    ks_f32[:, g, None].to_broadcast((P, P)), OP.mult)

            with tc.tile_pool(name="workB", bufs=3) as workB, \
                 tc.tile_pool(name="msgp", bufs=1) as msgp, \
                 tc.tile_pool(name="spool", bufs=3, space="PSUM") as spool, \
                 tc.tile_pool(name="mpool", bufs=3, space="PSUM") as mpool, \
                 tc.tile_pool(name="opool", bufs=2, space="PSUM") as opool:

                msgs = [msgp.tile([P, KT, CH], bf16, name=f"msg{c}")
                        for c in range(NCHUNK)]

                def merge(c):
                    for lt in range(CH // P):
                        o_ps = opool.tile([P, E], f32, name="o_ps")
                        for g in range(KT):
                            nc.tensor.matmul(
                                o_ps[:],
                                msgs[c][:, g, lt * P:(lt + 1) * P],
                                wm_sb[:, g, :],
                                start=(g == 0), stop=(g == KT - 1))
                        # ACT only: DVE paces phase B (recip + msg mults)
                        o_sb = workB.tile([P, E], f32, name="o_sb")
                        nc.scalar.copy(o_sb[:], o_ps[:])
                        nc.sync.dma_start(
                            out_d[c * CH + lt * P:c * CH + (lt + 1) * P, :],
                            o_sb[:])

                def emit_sw(c, g):
                    cs = slice(c * CH, (c + 1) * CH)
                    # replicated S: s_ps[p, l] = S[l, 4g + p//32]
                    s_ps = spool.tile([P, CH], f32, name="s_ps")
                    nc.tensor.matmul(
                        s_ps[:], bd_rep[:, g, :], qt_sb[:, g, cs],
                        start=True, stop=True)
                    # W = 1/(S/L + eps/L);  scale+bias on ACT, recip DVE
                    w_t = workB.tile([P, CH], f32, name="w_t", tag="w_t")
                    nc.scalar.activation(
                        w_t[:], s_ps[:], AFT.Copy,
                        bias=EPS / L, scale=1.0 / L)
                    w_r = workB.tile([P, CH], f32, name="w_r",
                                     tag=f"w_r{g % 2}")
                    nc.vector.reciprocal_approx_fast(w_r[:], w_t[:])
                    return w_r

                def emit_msg(c, g, w_r):
                    cs = slice(c * CH, (c + 1) * CH)
                    m_ps = mpool.tile([P, CH], f32, name="m_ps")
                    for j in range(KT):
                        sl32 = slice(32 * j, 32 * (j + 1))
                        nc.tensor.matmul(
                            m_ps[sl32, :],
                            red[sl32, 32 * g:32 * (g + 1)],
                            qt_sb[sl32, g, cs],
                            start=True, stop=True,
                            tile_position=(32 * j, 32 * j))
                    nc.vector.tensor_tensor(
                        msgs[c][:, g, :], m_ps[:], w_r[:], OP.mult)

                for c in range(NCHUNK):
                    if c < NCHUNK - 1:
                        for g in range(KT):
                            emit_msg(c, g, emit_sw(c, g))
                    else:
                        # last chunk: all S matmuls first so the W chains
                        # drain before the final msg mults -> merge starts
                        # sooner
                        wrs = [emit_sw(c, g) for g in range(KT)]
                        for g in range(KT):
                            emit_msg(c, g, wrs[g])
                    if c >= 1:
                        merge(c - 1)
                merge(NCHUNK - 1)

    nc.compile()
    return nc


def _get_nc():
    if "nc" not in _CACHE:
        _CACHE["nc"] = _build()
    return _CACHE["nc"]


def kernel(query, key, value, Wq, Wk, Wv, Wm):
    global LAST_EXEC_NS, LAST_RESULTS
    import os
    import ml_dtypes
    from concourse.bass_utils import run_bass_kernel_spmd

    bf = ml_dtypes.bfloat16
    query = np.asarray(query, dtype=np.float32)
    key = np.asarray(key, dtype=np.float32)
    value = np.asarray(value, dtype=np.float32)
    wq_t = np.ascontiguousarray(np.asarray(Wq, np.float32).T).astype(bf)
    wk_t = np.ascontiguousarray(np.asarray(Wk, np.float32).T).astype(bf)
    wv_t = np.ascontiguousarray(
        np.asarray(Wv, np.float32).T / L).astype(bf)
    wm_t = np.ascontiguousarray(np.asarray(Wm, np.float32).T).astype(bf)

    in_maps = []
    for c in range(NCORES):
        b, half = c // 2, c % 2
        rs = slice(half * R, (half + 1) * R)
        in_maps.append({
            "xq": np.ascontiguousarray(query[b, rs, :].T).astype(bf),
            "xk": np.ascontiguousarray(key[b, rs, :].T).astype(bf),
            "xv": np.ascontiguousarray(value[b, rs, :].T).astype(bf),
            "wq": wq_t, "wk": wk_t, "wv": wv_t, "wm": wm_t,
        })

    nc = _get_nc()
    trace = bool(int(os.environ.get("KERNEL_TRACE", "0")))
    res = run_bass_kernel_spmd(nc, in_maps, core_ids=list(range(NCORES)),
                               trace=trace)
    LAST_EXEC_NS = res.exec_time_ns
    LAST_RESULTS = res

    out = np.empty((B, L, E), dtype=np.float32)
    for c in range(NCORES):
        b, half = c // 2, c % 2
        out[b, half * R:(half + 1) * R, :] = res.results[c]["out"]
    return out
